# Initial kernel scaffold
#
"""Trainium2 Bass kernel for nn_DynamicWeightAttention.

Reference computation (per token t = (bt, n, h)):
    fused = concat(dyn[bt,n,h,:], static[n,h,:])            # C=32
    normed = LayerNorm(fused; gamma, beta, eps=1e-4)
    hmid   = tanh(normed @ w1 + b1)                         # HID=64
    score  = hmid @ w2 + b2                                 # scalar
    out[bt,n,:] = softmax over h of score                   # H=16

Strategy (8 NeuronCores, data-sharded over N: core c owns n in [32c, 32c+32)):
  - LayerNorm affine + static features fold host-side into per-n block
    weight matrices (static contribution enters via selector rows), so
    only the 16 dynamic features stream from HBM.
  - Per core, partition p owns bt-octet [8p, 8p+8); tokens stream
    through bf16 slabs of 128-column groups [64 dyn | 32 aux | 32 pad]
    that an xbar DMA transpose turns into feature-major rhs tiles with
    4 tokens per 128-row column.
  - Stats (mean/var) on DVE: bf16 sum-tree + exact segmented reduce of
    bf16 squares; invstd via bit-trick rsqrt + 2 Newton steps.
  - mm1: 2 bf16 K=128 matmuls per slab (zero-padded lhsT) -> h_pre for
    4 tokens/column in PSUM; tanh on ACT; mm2: 8 small accumulating
    matmuls fold w2 and regroup scores to [16=h, col=bt-octet];
    softmax: exp (ACT) -> ones-matmul denominators (PE) ->
    reciprocal_approx_fast (DVE) -> multiply (GPSIMD) -> xbar
    transpose to token-major -> cast-DMA out (one DMA per chunk).
  - Emission is software-pipelined: chunk c+1's DVE stats phase is
    emitted before chunk c's matmul phase so per-engine in-order
    streams overlap across chunks.
"""
import os

import numpy as np
import ml_dtypes

import concourse.bacc as bacc
import concourse.mybir as mybir
from concourse.ap import AP as BassAP
from concourse.tile import TileContext
from concourse.bass_utils import run_bass_kernel_spmd

F32 = mybir.dt.float32
BF16 = mybir.dt.bfloat16
U32 = mybir.dt.uint32
AT = mybir.AluOpType
AFT = mybir.ActivationFunctionType

B_T, N, H, PD, PS, HID = 1024, 256, 16, 16, 16, 64
NCORES = 8
NC_N = N // NCORES          # 32 n's per core
EPS = 1e-4
MAGIC = float(0x5F3759DF)

_cached = {}


def _host_prep(dynamic_features, static_features, ln_gamma, ln_beta, w1, b1, w2, b2):
    """Fold LN affine + static features into per-core packed weights."""
    g = np.asarray(ln_gamma, np.float32)
    be = np.asarray(ln_beta, np.float32)
    w1 = np.asarray(w1, np.float32)
    b1 = np.asarray(b1, np.float32)
    w2v = np.asarray(w2, np.float32).reshape(HID)
    st = np.asarray(static_features, np.float32)

    w1g = w1 * g[:, None]                      # [32, 64]
    w1d = w1g[:PD]                             # [16, 64] dyn part
    w1s = w1g[PD:]                             # [16, 64] static part
    cw = w1g.sum(0)                            # [64]
    b1p = b1 + be @ w1                         # [64]

    m2 = np.zeros((8, 128, 16), np.float32)
    for v in range(4):
        for half in range(2):
            m2[half * 4 + v, 0:64, 4 * v + 2 * half] = w2v
            m2[half * 4 + v, 64:128, 4 * v + 2 * half + 1] = w2v
    ones16 = np.ones((16, 16), np.float32)

    per_core = []
    for c in range(NCORES):
        stc = st[c * NC_N:(c + 1) * NC_N]      # [32, 16, 16]
        sp = np.einsum("nhp,pd->nhd", stc, w1s)  # [32, 16, 64]
        s_st = stc.sum(-1)                     # [32, 16]
        q_st = (stc ** 2).sum(-1)              # [32, 16]

        # K-row layout per slab column (r in [0,128)):
        #   r in [0,64):  dyn, a = r//16, f = r%16
        #   r in [64,96): aux, a = (r-64)//8, w = (r-64)%8:
        #                 w in 0..3 -> sel row v'=w (value inv iff v'==slab v),
        #                 w=4 -> mean*inv, w=5 -> const 1, w in 6..7 -> pad
        #   r in [96,128): pad
        wa = np.zeros((NC_N, 128, 128), np.float32)
        wb = np.zeros((NC_N, 128, 128), np.float32)
        for n in range(NC_N):
            for a in range(4):
                tgt = wa if a < 2 else wb
                mcol = 64 * (a % 2)
                tgt[n, 16 * a:16 * a + 16, mcol:mcol + 64] = w1d
                for vv in range(4):
                    tgt[n, 64 + 8 * a + vv, mcol:mcol + 64] = sp[n, 4 * vv + a]
                tgt[n, 64 + 8 * a + 4, mcol:mcol + 64] = -cw
                tgt[n, 64 + 8 * a + 5, mcol:mcol + 64] = b1p

        per_core.append({
            "dyn": np.ascontiguousarray(
                np.asarray(dynamic_features, np.float32)[:, c * NC_N:(c + 1) * NC_N]),
            "wa": wa.astype(ml_dtypes.bfloat16),
            "wb": wb.astype(ml_dtypes.bfloat16),
            "m2": m2.astype(ml_dtypes.bfloat16),
            "ones16": ones16.astype(ml_dtypes.bfloat16),
            "sst32": np.ascontiguousarray((s_st / 32.0).reshape(1, 512).astype(np.float32)),
            "qst32": np.ascontiguousarray((q_st / 32.0 + EPS).reshape(1, 512).astype(np.float32)),
        })
    return per_core


def build_nc(n_chunks=8):
    nc = bacc.Bacc("TRN2", target_bir_lowering=False, debug=False, num_devices=NCORES)
    dyn = nc.dram_tensor("dyn", [B_T, NC_N, H, PD], F32, kind="ExternalInput")
    wa_d = nc.dram_tensor("wa", [NC_N, 128, 128], BF16, kind="ExternalInput")
    wb_d = nc.dram_tensor("wb", [NC_N, 128, 128], BF16, kind="ExternalInput")
    m2_d = nc.dram_tensor("m2", [8, 128, 16], BF16, kind="ExternalInput")
    ones_d = nc.dram_tensor("ones16", [16, 16], BF16, kind="ExternalInput")
    sst_d = nc.dram_tensor("sst32", [1, 512], F32, kind="ExternalInput")
    qst_d = nc.dram_tensor("qst32", [1, 512], F32, kind="ExternalInput")
    out_d = nc.dram_tensor("out", [B_T, NC_N, H], F32, kind="ExternalOutput")

    dyn_v = dyn[:, :, :, :].rearrange("(p e) n h f -> p e n h f", e=8)
    NH = 16  # n's per half-chunk (staging granularity)

    with TileContext(nc) as tc:
        with tc.tile_pool(name="const", bufs=1) as cpool, \
             tc.tile_pool(name="stg", bufs=2) as stgpool, \
             tc.tile_pool(name="stats", bufs=1) as stpool, \
             tc.tile_pool(name="invp", bufs=2) as invpool, \
             tc.tile_pool(name="tr", bufs=2) as trpool, \
             tc.tile_pool(name="hid", bufs=2) as hpool, \
             tc.tile_pool(name="sm", bufs=2) as smpool, \
             tc.tile_pool(name="ot", bufs=2) as otpool, \
             tc.tile_pool(name="ps1", bufs=2, space="PSUM") as ps1pool, \
             tc.tile_pool(name="ps2", bufs=2, space="PSUM") as ps2pool, \
             tc.tile_pool(name="psd", bufs=2, space="PSUM") as psdpool:

            # ---- constants / weights (loaded once) ----
            wat = cpool.tile([128, NC_N, 128], BF16)
            nc.sync.dma_start(wat[:, :, :], wa_d[:, :, :].rearrange("n k m -> k n m"))
            wbt = cpool.tile([128, NC_N, 128], BF16)
            nc.sync.dma_start(wbt[:, :, :], wb_d[:, :, :].rearrange("n k m -> k n m"))
            m2t = cpool.tile([128, 8, 16], BF16)
            nc.sync.dma_start(m2t[:, :, :], m2_d[:, :, :].rearrange("s k m -> k s m"))
            onest = cpool.tile([16, 16], BF16)
            nc.sync.dma_start(onest[:, :], ones_d[:, :])
            sstt = cpool.tile([128, 512], F32)
            nc.sync.dma_start(sstt[0:1, :], sst_d[:, :])
            nc.gpsimd.partition_broadcast(sstt[:, :], sstt[0:1, :], channels=128)
            qstt = cpool.tile([128, 512], F32)
            nc.sync.dma_start(qstt[0:1, :], qst_d[:, :])
            nc.gpsimd.partition_broadcast(qstt[:, :], qstt[0:1, :], channels=128)

            # ---- persistent slab buffers (2, manually rotated) ----
            slabs = []
            for i in range(3):
                sl = cpool.tile([128, NC_N, 4, 128], BF16, tag=f"slab{i}")
                slf = sl[:, :, :, :].rearrange("p n v c -> p (n v c)")
                p0 = list(slf.ap)[0]
                nc.vector.memset(BassAP(slf.tensor, slf.offset + 70,
                                        [p0, [512, NC_N], [128, 4], [8, 4], [1, 2]]), 0.0)
                nc.vector.memset(BassAP(slf.tensor, slf.offset + 69,
                                        [p0, [512, NC_N], [128, 4], [8, 4]]), 1.0)
                nc.vector.memset(sl[:, :, :, 96:128], 0.0)
                slabs.append(sl)

            def stats_phase(b8):
                """Load chunk b8, compute invstd, fill slab aux + scaled dyn."""
                sl = slabs[b8 % 3]
                slf = sl[:, :, :, :].rearrange("p n v c -> p (n v c)")
                p0 = list(slf.ap)[0]
                ssum = stpool.tile([128, 512], F32, tag="ssum")
                q = stpool.tile([128, 512], F32, tag="q")
                stgs = []
                for hc in range(2):
                    n0 = hc * NH
                    stg = stgpool.tile([128, NH, H, PD], BF16, tag="stg")
                    stgs.append(stg)
                    nc.gpsimd.dma_start(stg[:, :, :, :], dyn_v[:, b8, n0:n0 + NH, :, :])
                    stg_f = stg[:, :, :, :].rearrange("p n h f -> p (n h) f")

                    t8 = stpool.tile([128, 256, 8], BF16, tag="t8")
                    nc.vector.tensor_tensor(t8[:, :, :], stg_f[:, :, 0:8], stg_f[:, :, 8:16], AT.add)
                    t4 = stpool.tile([128, 256, 4], BF16, tag="t4")
                    nc.vector.tensor_tensor(t4[:, :, :], t8[:, :, 0:4], t8[:, :, 4:8], AT.add)
                    t2 = stpool.tile([128, 256, 2], BF16, tag="t2")
                    nc.vector.tensor_tensor(t2[:, :, :], t4[:, :, 0:2], t4[:, :, 2:4], AT.add)
                    nc.vector.tensor_tensor(ssum[:, n0 * 16:(n0 + NH) * 16],
                                            t2[:, :, 0], t2[:, :, 1], AT.add)

                    x2 = stpool.tile([128, 256, 16], BF16, tag="t8")
                    nc.vector.tensor_tensor(x2[:, :, :], stg_f, stg_f, AT.mult)
                    nc.vector.tensor_reduce(q[:, n0 * 16:(n0 + NH) * 16], x2[:, :, :],
                                            axis=mybir.AxisListType.X, op=AT.add)

                # full-chunk stats chain [128, 512]
                mean = stpool.tile([128, 512], F32, tag="mean")
                nc.vector.scalar_tensor_tensor(mean[:, :], ssum[:, :], 1.0 / 32, sstt[:, :], AT.mult, AT.add)
                vareps = stpool.tile([128, 512], F32, tag="vareps")
                nc.vector.scalar_tensor_tensor(vareps[:, :], q[:, :], 1.0 / 32, qstt[:, :], AT.mult, AT.add)
                m2neg = stpool.tile([128, 512], F32, tag="m2neg")
                nc.vector.scalar_tensor_tensor(m2neg[:, :], mean[:, :], -1.0, mean[:, :], AT.mult, AT.mult)
                nc.vector.tensor_tensor(vareps[:, :], vareps[:, :], m2neg[:, :], AT.add)

                seed = stpool.tile([128, 512], U32, tag="q")
                nc.vector.tensor_scalar(seed[:, :], vareps[:, :].bitcast(U32), 1, None, AT.logical_shift_right)
                nc.vector.tensor_scalar(seed[:, :], seed[:, :], -1.0, MAGIC, AT.mult, AT.add)
                inv = invpool.tile([128, 512], F32, tag="inv")
                tmp = stpool.tile([128, 512], F32, tag="tmp")
                y0 = seed[:, :].bitcast(F32)
                nc.vector.tensor_tensor(tmp[:, :], y0, y0, AT.mult)
                nc.vector.scalar_tensor_tensor(tmp[:, :], tmp[:, :], -0.5, vareps[:, :], AT.mult, AT.mult)
                nc.vector.tensor_scalar(tmp[:, :], tmp[:, :], 1.5, None, AT.add)
                nc.vector.tensor_tensor(inv[:, :], y0, tmp[:, :], AT.mult)
                nc.vector.tensor_tensor(tmp[:, :], inv[:, :], inv[:, :], AT.mult)
                nc.vector.scalar_tensor_tensor(tmp[:, :], tmp[:, :], -0.5, vareps[:, :], AT.mult, AT.mult)
                nc.vector.tensor_scalar(tmp[:, :], tmp[:, :], 1.5, None, AT.add)
                nc.vector.tensor_tensor(inv[:, :], inv[:, :], tmp[:, :], AT.mult)

                minv = stpool.tile([128, 512], F32, tag="m2neg")
                nc.vector.tensor_tensor(minv[:, :], mean[:, :], inv[:, :], AT.mult)

                inv_nva = inv[:, :].rearrange("p (n v a) -> p n v a", n=NC_N, v=4)
                nc.vector.memset(BassAP(slf.tensor, slf.offset + 64,
                                        [p0, [512, NC_N], [128, 4], [8, 4], [1, 4]]), 0.0)
                nc.vector.tensor_copy(
                    BassAP(slf.tensor, slf.offset + 64,
                           [p0, [512, NC_N], [129, 4], [8, 4]]), inv_nva)
                nc.vector.tensor_copy(
                    BassAP(slf.tensor, slf.offset + 68,
                           [p0, [512, NC_N], [128, 4], [8, 4]]),
                    minv[:, :].rearrange("p (n v a) -> p n v a", n=NC_N, v=4))

                # scale dyn by invstd on GPSIMD (writes slab dyn region)
                for hc in range(2):
                    n0 = hc * NH
                    inv_h = (inv_nva[:, n0:n0 + NH, :, :]
                             .rearrange("p n v (a o) -> p n v a o", o=1)
                             .broadcast_to([128, NH, 4, 4, 16]))
                    nc.gpsimd.tensor_tensor(
                        sl[:, n0:n0 + NH, :, 0:64].rearrange("p n v (a f) -> p n v a f", a=4),
                        stgs[hc][:, :, :, :].rearrange("p n (v a) f -> p n v a f", v=4),
                        inv_h, AT.mult)

            def mm_phase(b8):
                """Transpose slab b8 and run mm1/tanh/mm2/softmax/output."""
                sl = slabs[b8 % 3]
                otc = otpool.tile([128, NC_N, H], BF16, tag="ot")
                for nb in range(8):
                    trt = trpool.tile([128, 16, 128], BF16, tag="tr")
                    nc.sync.dma_start_transpose(
                        trt[:, :, :],
                        sl[:, nb * 4:(nb + 1) * 4, :, :].rearrange("p n v c -> p (n v c)"))
                    for nl in range(4):
                        n = nb * 4 + nl
                        # psum layout: cols [0:512) = streams 0,1 (4 v's), [512:1024) = streams 2,3
                        ps = ps1pool.tile([128, 1024], F32, tag="ps1")
                        rhs4 = trt[:, nl * 4:nl * 4 + 4, :].rearrange("p s c -> p (s c)")
                        nc.tensor.matmul(ps[:, 0:512], wat[:, n, :], rhs4, start=True, stop=True)
                        nc.tensor.matmul(ps[:, 512:1024], wbt[:, n, :], rhs4, start=True, stop=True)
                        if n % 4 == 0:
                            ht4 = hpool.tile([128, 8, 4, 128], BF16, tag="h")
                            m2ps = ps2pool.tile([16, 512], F32, tag="ps2")
                        # ht4 free layout (s=half*4+v, j=n%4, c)
                        nc.scalar.activation(
                            ht4[:, :, n % 4, :],
                            ps[:, :].rearrange("p (s c) -> p s c", s=8), AFT.Tanh)
                        if n % 4 == 3:
                            for s in range(8):
                                nc.tensor.matmul(m2ps[:, :],
                                                 m2t[:, s, :],
                                                 ht4[:, s, :, :].rearrange("p j c -> p (j c)"),
                                                 start=(s == 0), stop=(s == 7))
                            et = smpool.tile([16, 512], BF16, tag="e")
                            nc.scalar.activation(et[:, :], m2ps[:, :], AFT.Exp)
                            dps = psdpool.tile([16, 512], F32, tag="psd")
                            nc.tensor.matmul(dps[:, :], onest[:, :], et[:, :], start=True, stop=True)
                            rt = smpool.tile([16, 512], F32, tag="r")
                            nc.vector.reciprocal_approx_fast(rt[:, :], dps[:, :])
                            ft = smpool.tile([16, 512], BF16, tag="f")
                            nc.gpsimd.tensor_tensor(ft[:, :], et[:, :], rt[:, :], AT.mult)
                            nc.sync.dma_start_transpose(otc[:, n - 3:n + 1, :], ft[:, :])
                # one output DMA per chunk: per-partition 2 KB contiguous runs
                nc.gpsimd.dma_start(
                    out_d[:, :, :].rearrange("(p e) n h -> p e n h", e=8)[:, b8, :, :],
                    otc[:, :, :])

            pend = [None]

            def softmax_tail(et, otc_t, n0t):
                dps = psdpool.tile([16, 512], F32, tag="psd")
                nc.tensor.matmul(dps[:, :], onest[:, :], et[:, :], start=True, stop=True)
                rt = smpool.tile([16, 512], F32, tag="r")
                nc.vector.reciprocal_approx_fast(rt[:, :], dps[:, :])
                ft = smpool.tile([16, 512], BF16, tag="f")
                nc.gpsimd.tensor_tensor(ft[:, :], et[:, :], rt[:, :], AT.mult)
                nc.sync.dma_start_transpose(otc_t[:, n0t:n0t + 4, :], ft[:, :])

            # software pipeline, depth 2: stats(c+2) emitted before mm(c)
            stats_phase(0)
            if n_chunks > 1:
                stats_phase(1)
            for b8 in range(n_chunks):
                if b8 + 2 < n_chunks:
                    stats_phase(b8 + 2)
                mm_phase(b8)
            if pend[0] is not None:
                softmax_tail(*pend[0])
    nc.compile()
    return nc


def kernel(**inputs):
    per_core = _host_prep(**inputs)
    if "nc" not in _cached:
        _cached["nc"] = build_nc()
    nc = _cached["nc"]
    trace = bool(os.environ.get("DWA_TRACE"))
    res = run_bass_kernel_spmd(nc, per_core, core_ids=list(range(NCORES)), trace=trace)
    if trace:
        print("HW exec time:", res.exec_time_ns, "ns")
        kernel.last_result = res
    out = np.empty((B_T, N, H), np.float32)
    for c in range(NCORES):
        out[:, c * NC_N:(c + 1) * NC_N, :] = res.results[c]["out"]
    return out



# revision 1
# speedup vs baseline: 1.0271x; 1.0271x over previous
"""Trainium2 Bass kernel for nn_DynamicWeightAttention.

Reference computation (per token t = (bt, n, h)):
    fused = concat(dyn[bt,n,h,:], static[n,h,:])            # C=32
    normed = LayerNorm(fused; gamma, beta, eps=1e-4)
    hmid   = tanh(normed @ w1 + b1)                         # HID=64
    score  = hmid @ w2 + b2                                 # scalar
    out[bt,n,:] = softmax over h of score                   # H=16

Strategy (8 NeuronCores, data-sharded over N: core c owns n in [32c, 32c+32)):
  - LayerNorm affine + static features fold host-side into per-n block
    weight matrices (static contribution enters via selector rows), so
    only the 16 dynamic features stream from HBM.
  - Per core, partition p owns bt-octet [8p, 8p+8); tokens stream
    through bf16 slabs of 128-column groups [64 dyn | 32 aux | 32 pad]
    that an xbar DMA transpose turns into feature-major rhs tiles with
    4 tokens per 128-row column.
  - Stats (mean/var) on DVE: bf16 sum-tree + exact segmented reduce of
    bf16 squares; invstd via bit-trick rsqrt + 2 Newton steps.
  - mm1: 2 bf16 K=128 matmuls per slab (zero-padded lhsT) -> h_pre for
    4 tokens/column in PSUM; tanh on ACT; mm2: 8 small accumulating
    matmuls fold w2 and regroup scores to [16=h, col=bt-octet];
    softmax: exp (ACT) -> ones-matmul denominators (PE) ->
    reciprocal_approx_fast (DVE) -> multiply (GPSIMD) -> xbar
    transpose to token-major -> cast-DMA out (one DMA per chunk).
  - Emission is software-pipelined: chunk c+1's DVE stats phase is
    emitted before chunk c's matmul phase so per-engine in-order
    streams overlap across chunks.
"""
import os

import numpy as np
import ml_dtypes

import concourse.bacc as bacc
import concourse.mybir as mybir
from concourse.ap import AP as BassAP
from concourse.tile import TileContext
from concourse.bass_utils import run_bass_kernel_spmd

F32 = mybir.dt.float32
BF16 = mybir.dt.bfloat16
U32 = mybir.dt.uint32
AT = mybir.AluOpType
AFT = mybir.ActivationFunctionType

B_T, N, H, PD, PS, HID = 1024, 256, 16, 16, 16, 64
NCORES = 8
NC_N = N // NCORES          # 32 n's per core
EPS = 1e-4
MAGIC = float(0x5F3759DF)

_cached = {}


def _host_prep(dynamic_features, static_features, ln_gamma, ln_beta, w1, b1, w2, b2):
    """Fold LN affine + static features into per-core packed weights."""
    g = np.asarray(ln_gamma, np.float32)
    be = np.asarray(ln_beta, np.float32)
    w1 = np.asarray(w1, np.float32)
    b1 = np.asarray(b1, np.float32)
    w2v = np.asarray(w2, np.float32).reshape(HID)
    st = np.asarray(static_features, np.float32)

    w1g = w1 * g[:, None]                      # [32, 64]
    w1d = w1g[:PD]                             # [16, 64] dyn part
    w1s = w1g[PD:]                             # [16, 64] static part
    cw = w1g.sum(0)                            # [64]
    b1p = b1 + be @ w1                         # [64]

    m2 = np.zeros((8, 128, 16), np.float32)
    for v in range(4):
        for half in range(2):
            m2[half * 4 + v, 0:64, 4 * v + 2 * half] = w2v
            m2[half * 4 + v, 64:128, 4 * v + 2 * half + 1] = w2v
    ones16 = np.ones((16, 16), np.float32)

    per_core = []
    for c in range(NCORES):
        stc = st[c * NC_N:(c + 1) * NC_N]      # [32, 16, 16]
        sp = np.einsum("nhp,pd->nhd", stc, w1s)  # [32, 16, 64]
        s_st = stc.sum(-1)                     # [32, 16]
        q_st = (stc ** 2).sum(-1)              # [32, 16]

        # K-row layout per slab column (r in [0,128)):
        #   r in [0,64):  dyn, a = r//16, f = r%16
        #   r in [64,96): aux, a = (r-64)//8, w = (r-64)%8:
        #                 w in 0..3 -> sel row v'=w (value inv iff v'==slab v),
        #                 w=4 -> mean*inv, w=5 -> const 1, w in 6..7 -> pad
        #   r in [96,128): pad
        wa = np.zeros((NC_N, 128, 128), np.float32)
        wb = np.zeros((NC_N, 128, 128), np.float32)
        for n in range(NC_N):
            for a in range(4):
                tgt = wa if a < 2 else wb
                mcol = 64 * (a % 2)
                tgt[n, 16 * a:16 * a + 16, mcol:mcol + 64] = w1d
                for vv in range(4):
                    tgt[n, 64 + 8 * a + vv, mcol:mcol + 64] = sp[n, 4 * vv + a]
                tgt[n, 64 + 8 * a + 4, mcol:mcol + 64] = -cw
                tgt[n, 64 + 8 * a + 5, mcol:mcol + 64] = b1p

        per_core.append({
            "dyn": np.ascontiguousarray(
                np.asarray(dynamic_features, np.float32)[:, c * NC_N:(c + 1) * NC_N]),
            "wa": wa.astype(ml_dtypes.bfloat16),
            "wb": wb.astype(ml_dtypes.bfloat16),
            "m2": m2.astype(ml_dtypes.bfloat16),
            "ones16": ones16.astype(ml_dtypes.bfloat16),
            "sst32": np.ascontiguousarray((s_st / 32.0).reshape(1, 512).astype(np.float32)),
            "qst32": np.ascontiguousarray((q_st / 32.0 + EPS).reshape(1, 512).astype(np.float32)),
        })
    return per_core


def build_nc(n_chunks=8):
    nc = bacc.Bacc("TRN2", target_bir_lowering=False, debug=False, num_devices=NCORES)
    dyn = nc.dram_tensor("dyn", [B_T, NC_N, H, PD], F32, kind="ExternalInput")
    wa_d = nc.dram_tensor("wa", [NC_N, 128, 128], BF16, kind="ExternalInput")
    wb_d = nc.dram_tensor("wb", [NC_N, 128, 128], BF16, kind="ExternalInput")
    m2_d = nc.dram_tensor("m2", [8, 128, 16], BF16, kind="ExternalInput")
    ones_d = nc.dram_tensor("ones16", [16, 16], BF16, kind="ExternalInput")
    sst_d = nc.dram_tensor("sst32", [1, 512], F32, kind="ExternalInput")
    qst_d = nc.dram_tensor("qst32", [1, 512], F32, kind="ExternalInput")
    out_d = nc.dram_tensor("out", [B_T, NC_N, H], F32, kind="ExternalOutput")

    dyn_v = dyn[:, :, :, :].rearrange("(p e) n h f -> p e n h f", e=8)
    NH = 16  # n's per half-chunk (staging granularity)

    with TileContext(nc) as tc:
        with tc.tile_pool(name="const", bufs=1) as cpool, \
             tc.tile_pool(name="stg", bufs=2) as stgpool, \
             tc.tile_pool(name="stats", bufs=1) as stpool, \
             tc.tile_pool(name="invp", bufs=2) as invpool, \
             tc.tile_pool(name="tr", bufs=2) as trpool, \
             tc.tile_pool(name="hid", bufs=2) as hpool, \
             tc.tile_pool(name="sm", bufs=2) as smpool, \
             tc.tile_pool(name="ot", bufs=2) as otpool, \
             tc.tile_pool(name="ps1", bufs=2, space="PSUM") as ps1pool, \
             tc.tile_pool(name="ps2", bufs=2, space="PSUM") as ps2pool, \
             tc.tile_pool(name="psd", bufs=2, space="PSUM") as psdpool:

            # ---- constants / weights (loaded once) ----
            wat = cpool.tile([128, NC_N, 128], BF16)
            nc.sync.dma_start(wat[:, :, :], wa_d[:, :, :].rearrange("n k m -> k n m"))
            wbt = cpool.tile([128, NC_N, 128], BF16)
            nc.sync.dma_start(wbt[:, :, :], wb_d[:, :, :].rearrange("n k m -> k n m"))
            m2t = cpool.tile([128, 8, 16], BF16)
            nc.sync.dma_start(m2t[:, :, :], m2_d[:, :, :].rearrange("s k m -> k s m"))
            onest = cpool.tile([16, 16], BF16)
            nc.sync.dma_start(onest[:, :], ones_d[:, :])
            sstt = cpool.tile([128, 512], F32)
            nc.sync.dma_start(sstt[0:1, :], sst_d[:, :])
            nc.gpsimd.partition_broadcast(sstt[:, :], sstt[0:1, :], channels=128)
            qstt = cpool.tile([128, 512], F32)
            nc.sync.dma_start(qstt[0:1, :], qst_d[:, :])
            nc.gpsimd.partition_broadcast(qstt[:, :], qstt[0:1, :], channels=128)

            # ---- persistent slab buffers (2, manually rotated) ----
            slabs = []
            for i in range(3):
                sl = cpool.tile([128, NC_N, 4, 128], BF16, tag=f"slab{i}")
                slf = sl[:, :, :, :].rearrange("p n v c -> p (n v c)")
                p0 = list(slf.ap)[0]
                nc.vector.memset(BassAP(slf.tensor, slf.offset + 70,
                                        [p0, [512, NC_N], [128, 4], [8, 4], [1, 2]]), 0.0)
                nc.vector.memset(BassAP(slf.tensor, slf.offset + 69,
                                        [p0, [512, NC_N], [128, 4], [8, 4]]), 1.0)
                nc.vector.memset(sl[:, :, :, 96:128], 0.0)
                slabs.append(sl)

            def stats_phase(b8):
                """Load chunk b8, compute invstd, fill slab aux + scaled dyn."""
                sl = slabs[b8 % 3]
                slf = sl[:, :, :, :].rearrange("p n v c -> p (n v c)")
                p0 = list(slf.ap)[0]
                ssum = stpool.tile([128, 512], F32, tag="ssum")
                q = stpool.tile([128, 512], F32, tag="q")
                stgs = []
                for hc in range(2):
                    n0 = hc * NH
                    stg = stgpool.tile([128, NH, H, PD], BF16, tag="stg")
                    stgs.append(stg)
                    nc.gpsimd.dma_start(stg[:, :, :, :], dyn_v[:, b8, n0:n0 + NH, :, :])
                    stg_f = stg[:, :, :, :].rearrange("p n h f -> p (n h) f")

                    t8 = stpool.tile([128, 256, 8], BF16, tag="t8")
                    nc.vector.tensor_tensor(t8[:, :, :], stg_f[:, :, 0:8], stg_f[:, :, 8:16], AT.add)
                    t4 = stpool.tile([128, 256, 4], BF16, tag="t4")
                    nc.vector.tensor_tensor(t4[:, :, :], t8[:, :, 0:4], t8[:, :, 4:8], AT.add)
                    t2 = stpool.tile([128, 256, 2], BF16, tag="t2")
                    nc.vector.tensor_tensor(t2[:, :, :], t4[:, :, 0:2], t4[:, :, 2:4], AT.add)
                    nc.vector.tensor_tensor(ssum[:, n0 * 16:(n0 + NH) * 16],
                                            t2[:, :, 0], t2[:, :, 1], AT.add)

                    x2 = stpool.tile([128, 256, 16], BF16, tag="t8")
                    nc.vector.tensor_tensor(x2[:, :, :], stg_f, stg_f, AT.mult)
                    nc.vector.tensor_reduce(q[:, n0 * 16:(n0 + NH) * 16], x2[:, :, :],
                                            axis=mybir.AxisListType.X, op=AT.add)

                # full-chunk stats chain [128, 512]
                mean = stpool.tile([128, 512], F32, tag="mean")
                nc.vector.scalar_tensor_tensor(mean[:, :], ssum[:, :], 1.0 / 32, sstt[:, :], AT.mult, AT.add)
                vareps = stpool.tile([128, 512], F32, tag="vareps")
                nc.vector.scalar_tensor_tensor(vareps[:, :], q[:, :], 1.0 / 32, qstt[:, :], AT.mult, AT.add)
                m2neg = stpool.tile([128, 512], F32, tag="m2neg")
                nc.vector.scalar_tensor_tensor(m2neg[:, :], mean[:, :], -1.0, mean[:, :], AT.mult, AT.mult)
                nc.vector.tensor_tensor(vareps[:, :], vareps[:, :], m2neg[:, :], AT.add)

                seed = stpool.tile([128, 512], U32, tag="q")
                nc.vector.tensor_scalar(seed[:, :], vareps[:, :].bitcast(U32), 1, None, AT.logical_shift_right)
                nc.vector.tensor_scalar(seed[:, :], seed[:, :], -1.0, MAGIC, AT.mult, AT.add)
                inv = invpool.tile([128, 512], F32, tag="inv")
                tmp = stpool.tile([128, 512], F32, tag="tmp")
                y0 = seed[:, :].bitcast(F32)
                nc.vector.tensor_tensor(tmp[:, :], y0, y0, AT.mult)
                nc.vector.scalar_tensor_tensor(tmp[:, :], tmp[:, :], -0.5, vareps[:, :], AT.mult, AT.mult)
                nc.vector.tensor_scalar(tmp[:, :], tmp[:, :], 1.5, None, AT.add)
                nc.vector.tensor_tensor(inv[:, :], y0, tmp[:, :], AT.mult)
                nc.vector.tensor_tensor(tmp[:, :], inv[:, :], inv[:, :], AT.mult)
                nc.vector.scalar_tensor_tensor(tmp[:, :], tmp[:, :], -0.5, vareps[:, :], AT.mult, AT.mult)
                nc.vector.tensor_scalar(tmp[:, :], tmp[:, :], 1.5, None, AT.add)
                nc.vector.tensor_tensor(inv[:, :], inv[:, :], tmp[:, :], AT.mult)

                minv = stpool.tile([128, 512], F32, tag="m2neg")
                nc.vector.tensor_tensor(minv[:, :], mean[:, :], inv[:, :], AT.mult)

                inv_nva = inv[:, :].rearrange("p (n v a) -> p n v a", n=NC_N, v=4)
                nc.vector.memset(BassAP(slf.tensor, slf.offset + 64,
                                        [p0, [512, NC_N], [128, 4], [8, 4], [1, 4]]), 0.0)
                nc.vector.tensor_copy(
                    BassAP(slf.tensor, slf.offset + 64,
                           [p0, [512, NC_N], [129, 4], [8, 4]]), inv_nva)
                nc.vector.tensor_copy(
                    BassAP(slf.tensor, slf.offset + 68,
                           [p0, [512, NC_N], [128, 4], [8, 4]]),
                    minv[:, :].rearrange("p (n v a) -> p n v a", n=NC_N, v=4))

                # scale dyn by invstd on GPSIMD (writes slab dyn region)
                for hc in range(2):
                    n0 = hc * NH
                    inv_h = (inv_nva[:, n0:n0 + NH, :, :]
                             .rearrange("p n v (a o) -> p n v a o", o=1)
                             .broadcast_to([128, NH, 4, 4, 16]))
                    nc.gpsimd.tensor_tensor(
                        sl[:, n0:n0 + NH, :, 0:64].rearrange("p n v (a f) -> p n v a f", a=4),
                        stgs[hc][:, :, :, :].rearrange("p n (v a) f -> p n v a f", v=4),
                        inv_h, AT.mult)

            def mm_phase(b8):
                """Transpose slab b8 and run mm1/tanh/mm2/softmax/output."""
                sl = slabs[b8 % 3]
                otc = otpool.tile([128, NC_N, H], BF16, tag="ot")
                for nb in range(8):
                    trt = trpool.tile([128, 16, 128], BF16, tag="tr")
                    nc.sync.dma_start_transpose(
                        trt[:, :, :],
                        sl[:, nb * 4:(nb + 1) * 4, :, :].rearrange("p n v c -> p (n v c)"))
                    for nl in range(4):
                        n = nb * 4 + nl
                        # psum layout: cols [0:512) = streams 0,1 (4 v's), [512:1024) = streams 2,3
                        ps = ps1pool.tile([128, 1024], F32, tag="ps1")
                        rhs4 = trt[:, nl * 4:nl * 4 + 4, :].rearrange("p s c -> p (s c)")
                        nc.tensor.matmul(ps[:, 0:512], wat[:, n, :], rhs4, start=True, stop=True)
                        nc.tensor.matmul(ps[:, 512:1024], wbt[:, n, :], rhs4, start=True, stop=True)
                        if n % 4 == 0:
                            ht4 = hpool.tile([128, 8, 4, 128], BF16, tag="h")
                            m2ps = ps2pool.tile([16, 512], F32, tag="ps2")
                        # ht4 free layout (s=half*4+v, j=n%4, c)
                        nc.scalar.activation(
                            ht4[:, :, n % 4, :],
                            ps[:, :].rearrange("p (s c) -> p s c", s=8), AFT.Tanh)
                        if n % 4 == 3:
                            for s in range(8):
                                nc.tensor.matmul(m2ps[:, :],
                                                 m2t[:, s, :],
                                                 ht4[:, s, :, :].rearrange("p j c -> p (j c)"),
                                                 start=(s == 0), stop=(s == 7))
                            et = smpool.tile([16, 512], BF16, tag="e")
                            nc.scalar.activation(et[:, :], m2ps[:, :], AFT.Exp)
                            dps = psdpool.tile([16, 512], F32, tag="psd")
                            nc.tensor.matmul(dps[:, :], onest[:, :], et[:, :], start=True, stop=True)
                            rt = smpool.tile([16, 512], F32, tag="r")
                            nc.vector.reciprocal_approx_fast(rt[:, :], dps[:, :])
                            ft = smpool.tile([16, 512], BF16, tag="f")
                            nc.gpsimd.tensor_tensor(ft[:, :], et[:, :], rt[:, :], AT.mult)
                            nc.sync.dma_start_transpose(otc[:, n - 3:n + 1, :], ft[:, :])
                # one output DMA per chunk: per-partition 2 KB contiguous runs
                nc.gpsimd.dma_start(
                    out_d[:, :, :].rearrange("(p e) n h -> p e n h", e=8)[:, b8, :, :],
                    otc[:, :, :])

            pend = [None]

            def softmax_tail(et, otc_t, n0t):
                dps = psdpool.tile([16, 512], F32, tag="psd")
                nc.tensor.matmul(dps[:, :], onest[:, :], et[:, :], start=True, stop=True)
                rt = smpool.tile([16, 512], F32, tag="r")
                nc.vector.reciprocal_approx_fast(rt[:, :], dps[:, :])
                ft = smpool.tile([16, 512], BF16, tag="f")
                nc.gpsimd.tensor_tensor(ft[:, :], et[:, :], rt[:, :], AT.mult)
                nc.sync.dma_start_transpose(otc_t[:, n0t:n0t + 4, :], ft[:, :])

            # software pipeline, depth 2: stats(c+2) emitted before mm(c)
            stats_phase(0)
            if n_chunks > 1:
                stats_phase(1)
            for b8 in range(n_chunks):
                if b8 + 2 < n_chunks:
                    stats_phase(b8 + 2)
                mm_phase(b8)
            if pend[0] is not None:
                softmax_tail(*pend[0])
    nc.compile()
    return nc


def kernel(**inputs):
    per_core = _host_prep(**inputs)
    if "nc" not in _cached:
        _cached["nc"] = build_nc()
    nc = _cached["nc"]
    trace = bool(os.environ.get("DWA_TRACE"))
    res = run_bass_kernel_spmd(nc, per_core, core_ids=list(range(NCORES)), trace=trace)
    if trace:
        print("HW exec time:", res.exec_time_ns, "ns")
        kernel.last_result = res
    out = np.empty((B_T, N, H), np.float32)
    for c in range(NCORES):
        out[:, c * NC_N:(c + 1) * NC_N, :] = res.results[c]["out"]
    return out

